# revision 6
# baseline (speedup 1.0000x reference)
# Trainium2 Bass kernel for ClassAttn (single class-token query attention).
#
# Math (per batch b):
#   q   = x[b,0] @ Wq * scale                       [CR]
#   logits[h,n] = sum_d q[h,d] * (x[b] @ Wk)[n,h,d]
#               = sum_c x[b,n,c] * wq_eff[c,h]      with wq_eff[c,h] = sum_d Wk[c,h*HD+d] q[h*HD+d]
#   w = exp(logits)          (inputs are bounded; softmax needs no max-subtraction)
#   z[h] = sum_n w[h,n]
#   s[h,c] = sum_n w[h,n] x[b,n,c]                  (attn-weighted token sum)
#   o[h,d] = (1/z[h]) sum_c s[h,c] Wv[c,h*HD+d]
#   out = o.flatten() @ Wp + bp
#
# This avoids materializing K and V entirely: the heavy work is two
# [N,C]-sized streaming contractions (logits and s) per batch instead of two
# [N,C]x[C,CR] projections — 16x fewer FLOPs.
#
# Sharding: data-parallel over batch. 8 cores x 4 batches each; weights
# replicated; no collectives. Per-core x shard is streamed in groups of 512
# tokens, cast fp32->bf16 in the DMA (SWDGE cast), transposed on the PE
# (needed because the logits contraction is over c, which must live on
# partitions), and consumed twice (logits from x^T, s-accum from x natural).

import numpy as np
from contextlib import ExitStack

import concourse.bass as bass
import concourse.mybir as mybir
import concourse.tile as tile
from concourse import bacc
from concourse.masks import make_identity

F32 = mybir.dt.float32
BF16 = mybir.dt.bfloat16

B, N, C = 32, 4096, 1024
H, HD = 16, 16
CR = 256
SCALE = HD ** -0.5
NCORES = 8
BS = B // NCORES          # batches per core
GTOK = 512                # tokens per group
BLK = 128                 # tokens per block (partition tile)
NBLK = GTOK // BLK        # 4 blocks per group
NCB = C // 128            # 8 c-blocks


def emit(tc, x_d, wq_d, wk_d, wv_d, wp_d, bp_d, dmask_d, out_d, bs, n):
    nc = tc.nc
    ngroups = n // GTOK
    with ExitStack() as ctx:
        const = ctx.enter_context(tc.tile_pool(name="const", bufs=1))
        px = ctx.enter_context(tc.tile_pool(name="px", bufs=2))
        pxt = ctx.enter_context(tc.tile_pool(name="pxt", bufs=2))
        pw = ctx.enter_context(tc.tile_pool(name="pw", bufs=2))
        pb = ctx.enter_context(tc.tile_pool(name="pb", bufs=2))
        ps_xt = ctx.enter_context(tc.tile_pool(name="ps_xt", bufs=3, space="PSUM"))
        ps_sm = ctx.enter_context(tc.tile_pool(name="ps_sm", bufs=2, space="PSUM"))
        ps_s = ctx.enter_context(tc.tile_pool(name="ps_s", bufs=1, space="PSUM"))
        ps_z = ctx.enter_context(tc.tile_pool(name="ps_z", bufs=1, space="PSUM"))

        # ---- constants / weights ----
        ident = const.tile([128, 128], BF16)
        make_identity(nc, ident[:])
        ones_col = const.tile([128, 1], BF16)
        nc.vector.memset(ones_col[:], 1.0)
        ones_row = const.tile([1, 128], BF16)
        nc.vector.memset(ones_row[:], 1.0)

        wq_sb = const.tile([128, NCB, CR], BF16)     # Wq[c,r] c-blocked, bf16
        nc.gpsimd.dma_start(out=wq_sb[:], in_=wq_d.rearrange("(j p) r -> p j r", p=128))
        wk_sb = const.tile([128, NCB, CR], F32)      # Wk[c,r] fp32 (for wq_eff)
        nc.sync.dma_start(out=wk_sb[:], in_=wk_d.rearrange("(j p) r -> p j r", p=128))
        wv_sb = const.tile([128, NCB, CR], BF16)     # Wv[c,r]
        nc.gpsimd.dma_start(out=wv_sb[:], in_=wv_d.rearrange("(j p) r -> p j r", p=128))
        wp_sb = const.tile([128, 2, C], BF16)        # Wp[r,c] r-blocked
        nc.gpsimd.dma_start(out=wp_sb[:], in_=wp_d.rearrange("(j p) c -> p j c", p=128))
        bp_sb = const.tile([1, C], F32)
        nc.sync.dma_start(out=bp_sb[:], in_=bp_d.rearrange("(u c) -> u c", u=1))
        dmask_sb = const.tile([128, 2, H], F32)   # dmask[p,half,h] = (h == 8*half + p//16)
        nc.sync.dma_start(out=dmask_sb[:], in_=dmask_d)

        for b in range(bs):
            s_ps = ps_s.tile([16, C], F32, tag="s")          # 2 banks
            z_ps = ps_z.tile([16, 1], F32, tag="z")          # 1 bank
            wq_eff_bf = None

            for g in range(ngroups):
                # ---- load + cast one group of 512 tokens ----
                xg = px.tile([128, NBLK, C], BF16, tag="xg")
                for blk in range(NBLK):
                    nc.gpsimd.dma_start(
                        out=xg[:, blk, :],
                        in_=x_d[b, g * GTOK + blk * BLK : g * GTOK + (blk + 1) * BLK, :],
                    )
                # ---- transpose x -> xT (c on partitions) ----
                xt = pxt.tile([128, NCB, GTOK], BF16, tag="xt")
                for blk in range(NBLK):
                    for j in range(NCB):
                        xt_ps = ps_xt.tile([128, 128], BF16, tag="xt_ps")
                        nc.tensor.transpose(
                            xt_ps[:], xg[:, blk, j * 128 : (j + 1) * 128], ident[:]
                        )
                        nc.vector.tensor_copy(
                            xt[:, j, blk * BLK : (blk + 1) * BLK], xt_ps[:]
                        )

                if g == 0:
                    # ---- per-batch prologue: q, wq_eff ----
                    q_ps = ps_sm.tile([1, CR], F32, tag="sm")
                    for j in range(NCB):
                        nc.tensor.matmul(
                            q_ps[:], xt[:, j, 0:1], wq_sb[:, j, :],
                            start=(j == 0), stop=(j == NCB - 1),
                        )
                    qs_bf = pb.tile([1, CR], BF16, tag="qs")
                    nc.scalar.mul(qs_bf[:], q_ps[:], SCALE)
                    rep_ps = ps_sm.tile([128, CR], F32, tag="sm")
                    nc.tensor.matmul(rep_ps[:], ones_row[:], qs_bf[:])
                    qs_rep = pb.tile([128, CR], F32, tag="qs_rep")
                    nc.vector.tensor_copy(qs_rep[:], rep_ps[:])
                    wq_eff = pb.tile([128, NCB, H], F32, tag="wq_eff")
                    tmp = pb.tile([128, CR], F32, tag="tmp")
                    for j in range(NCB):
                        nc.vector.tensor_mul(tmp[:], wk_sb[:, j, :], qs_rep[:])
                        nc.vector.reduce_sum(
                            wq_eff[:, j, :],
                            tmp.rearrange("p (h d) -> p h d", h=H),
                            axis=mybir.AxisListType.X,
                        )
                    wq_eff_bf = pb.tile([128, NCB, H], BF16, tag="wq_eff_bf")
                    nc.vector.tensor_copy(wq_eff_bf[:], wq_eff[:])

                # ---- logits^T [16, 512] ----
                lg_ps = ps_sm.tile([16, GTOK], F32, tag="sm")
                for j in range(NCB):
                    nc.tensor.matmul(
                        lg_ps[:], wq_eff_bf[:, j, :], xt[:, j, :],
                        start=(j == 0), stop=(j == NCB - 1),
                    )
                # ---- w = exp(logits) ----
                wT = pw.tile([16, GTOK], BF16, tag="wT")
                nc.scalar.activation(wT[:], lg_ps[:], mybir.ActivationFunctionType.Exp)
                # ---- transpose w back to [n, 16]; accumulate z and s ----
                w_sb = pw.tile([128, NBLK, H], BF16, tag="w_sb")
                for blk in range(NBLK):
                    w_ps = ps_sm.tile([128, H], BF16, tag="sm")
                    nc.tensor.transpose(
                        w_ps[:], wT[:, blk * BLK : (blk + 1) * BLK], ident[:16, :16]
                    )
                    nc.vector.tensor_copy(w_sb[:, blk, :], w_ps[:])
                    first = g == 0 and blk == 0
                    last = g == ngroups - 1 and blk == NBLK - 1
                    nc.tensor.matmul(
                        z_ps[:], w_sb[:, blk, :], ones_col[:],
                        start=first, stop=last,
                    )
                    for half in range(2):
                        nc.tensor.matmul(
                            s_ps[:, half * 512 : (half + 1) * 512],
                            w_sb[:, blk, :],
                            xg[:, blk, half * 512 : (half + 1) * 512],
                            start=first, stop=last,
                        )

            # ---- per-batch epilogue ----
            rz = pb.tile([16, 1], F32, tag="rz")
            nc.vector.reciprocal(rz[:], z_ps[:])
            sbar = pb.tile([16, C], BF16, tag="sbar")
            nc.vector.tensor_scalar_mul(sbar[:], s_ps[:], rz[:])
            stT = pb.tile([128, NCB, H], BF16, tag="stT")
            for j in range(NCB):
                st_ps = ps_sm.tile([128, H], BF16, tag="sm")
                nc.tensor.transpose(
                    st_ps[:], sbar[:, j * 128 : (j + 1) * 128], ident[:16, :16]
                )
                nc.vector.tensor_copy(stT[:, j, :], st_ps[:])
            # o_fullT[cr, h] = sum_c Wv[c, cr] * sbar[h, c] ; keep only h == cr//HD
            o_flatT_f = pb.tile([128, 2], F32, tag="o_flatT_f")
            o_flatT = pb.tile([128, 2], BF16, tag="o_flatT")
            for half in range(2):
                of_ps = ps_sm.tile([128, H], F32, tag="sm")
                for j in range(NCB):
                    nc.tensor.matmul(
                        of_ps[:], wv_sb[:, j, half * 128 : (half + 1) * 128],
                        stT[:, j, :],
                        start=(j == 0), stop=(j == NCB - 1),
                    )
                om = pb.tile([128, H], F32, tag="om")
                nc.vector.tensor_mul(om[:], of_ps[:], dmask_sb[:, half, :])
                nc.vector.reduce_sum(
                    o_flatT_f[:, half : half + 1], om[:], axis=mybir.AxisListType.X
                )
            nc.vector.tensor_copy(o_flatT[:], o_flatT_f[:])
            # out = o_flat @ Wp + bp
            out_sb = pb.tile([1, C], F32, tag="out_sb")
            for half in range(2):
                op_ps = ps_sm.tile([1, 512], F32, tag="sm")
                for j in range(2):
                    nc.tensor.matmul(
                        op_ps[:], o_flatT[:, j : j + 1],
                        wp_sb[:, j, half * 512 : (half + 1) * 512],
                        start=(j == 0), stop=(j == 1),
                    )
                nc.vector.tensor_add(
                    out_sb[:, half * 512 : (half + 1) * 512], op_ps[:],
                    bp_sb[:, half * 512 : (half + 1) * 512],
                )
            nc.sync.dma_start(out=out_d[b : b + 1, :], in_=out_sb[:])


def make_dmask():
    dm = np.zeros((128, 2, H), dtype=np.float32)
    for p in range(128):
        for half in range(2):
            dm[p, half, 8 * half + p // 16] = 1.0
    return dm


def build_bass(bs=BS, n=N):
    nc = bacc.Bacc("TRN2", target_bir_lowering=False, debug=False, num_devices=NCORES)
    x_d = nc.dram_tensor("x", [bs, n, C], F32, kind="ExternalInput").ap()
    wq_d = nc.dram_tensor("Wq", [C, CR], F32, kind="ExternalInput").ap()
    wk_d = nc.dram_tensor("Wk", [C, CR], F32, kind="ExternalInput").ap()
    wv_d = nc.dram_tensor("Wv", [C, CR], F32, kind="ExternalInput").ap()
    wp_d = nc.dram_tensor("Wp", [CR, C], F32, kind="ExternalInput").ap()
    bp_d = nc.dram_tensor("bp", [C], F32, kind="ExternalInput").ap()
    dmask_d = nc.dram_tensor("dmask", [128, 2, H], F32, kind="ExternalInput").ap()
    out_d = nc.dram_tensor("out", [bs, C], F32, kind="ExternalOutput").ap()
    with tile.TileContext(nc) as tc:
        emit(tc, x_d, wq_d, wk_d, wv_d, wp_d, bp_d, dmask_d, out_d, bs, n)
    nc.compile()
    return nc


def kernel(**inputs):
    from concourse.bass_utils import run_bass_kernel_spmd

    x = np.ascontiguousarray(np.asarray(inputs["x"], dtype=np.float32))
    wq = np.ascontiguousarray(np.asarray(inputs["Wq"], dtype=np.float32))
    wk = np.ascontiguousarray(np.asarray(inputs["Wk"], dtype=np.float32))
    wv = np.ascontiguousarray(np.asarray(inputs["Wv"], dtype=np.float32))
    wp = np.ascontiguousarray(np.asarray(inputs["Wp"], dtype=np.float32))
    bp = np.ascontiguousarray(np.asarray(inputs["bp"], dtype=np.float32))

    nc = build_bass()
    dmask = make_dmask()
    in_maps = [
        {
            "x": x[c * BS : (c + 1) * BS],
            "Wq": wq, "Wk": wk, "Wv": wv, "Wp": wp, "bp": bp,
            "dmask": dmask,
        }
        for c in range(NCORES)
    ]
    res = run_bass_kernel_spmd(nc, in_maps, core_ids=list(range(NCORES)))
    out = np.concatenate([r["out"] for r in res.results], axis=0)  # [B, C]
    return out.reshape(B, 1, C).astype(np.float32)
